# revision 1
# baseline (speedup 1.0000x reference)
"""CrossAttentionBlock kernel for 8 Trainium2 NeuronCores.

Reference computation (per batch b):
    q = x @ Wq;  k,v = y @ Wkv;  per head: softmax(q k^T / sqrt(dk)) v;
    out = concat_heads @ Wproj + bproj

Sharding: 8 cores = 2 batches x 4 head-groups (4 heads each). Each core
computes the partial output contribution of its 4 heads for its batch;
the host sums the 4 partials per batch and adds the bias.

Per-core layout (host prepares):
    xT  [1024, 2048]  x[b].T            (contraction dim on partitions)
    yT  [768, 2048]   y[b].T
    wq  [1024, 256]   Wq columns of this head group
    wk  [768, 256]    K-half of Wkv for this head group
    wv  [768, 256]    V-half of Wkv for this head group
    wp  [256, 1024]   Wproj rows of this head group
Output:
    outT [1024, 2048] partial (x @ .. @ Wproj).T for this head group

All matmuls run in float32r (full PE rate). PSUM accumulation is fp32.
"""

import numpy as np

import concourse.bass as bass
import concourse.tile as tile
from concourse import bacc, mybir
from concourse.bass_utils import run_bass_kernel_spmd

B, LQ, LKV = 2, 2048, 2048
C, CTX, H, DK = 1024, 768, 16, 64
SCALE = DK ** (-0.5)

F32 = mybir.dt.float32
F32R = mybir.dt.float32r


def _bcast_rows(ap: bass.AP, nrows: int) -> bass.AP:
    """AP that reads a single-partition row `nrows` times (partition step 0)."""
    assert ap.ap[0][1] == 1, ap.ap
    return bass.AP(tensor=ap.tensor, offset=ap.offset, ap=[[0, nrows]] + ap.ap[1:])


def build_kernel(lq=LQ, lkv=LKV, c=C, ctx=CTX, hd=256, debug_taps=False):
    """One core's program: 4 heads (2 pairs) of cross-attention + partial proj."""
    nc = bacc.Bacc("TRN2", target_bir_lowering=False, debug=False)

    xT = nc.dram_tensor("xT", [c, lq], F32, kind="ExternalInput").ap()
    yT = nc.dram_tensor("yT", [ctx, lkv], F32, kind="ExternalInput").ap()
    wq = nc.dram_tensor("wq", [c, hd], F32, kind="ExternalInput").ap()
    wk = nc.dram_tensor("wk", [ctx, hd], F32, kind="ExternalInput").ap()
    wv = nc.dram_tensor("wv", [ctx, hd], F32, kind="ExternalInput").ap()
    wp = nc.dram_tensor("wp", [hd, c], F32, kind="ExternalInput").ap()
    outT = nc.dram_tensor("outT", [c, lq], F32, kind="ExternalOutput").ap()
    # DRAM bounce buffer for the per-row 1/rowsum broadcast (SBUF APs cannot
    # have partition step 0; DRAM APs can)
    rsd = nc.dram_tensor("rsd", [hd // 128, lq // 512, 2, 512], F32,
                         kind="Internal").ap()
    taps = {}
    if debug_taps:
        taps["dbg_qt"] = nc.dram_tensor(
            "dbg_qt", [128, hd // 128, lq], F32, kind="ExternalOutput").ap()
        taps["dbg_kt"] = nc.dram_tensor(
            "dbg_kt", [128, hd // 128, lkv], F32, kind="ExternalOutput").ap()
        taps["dbg_vaug"] = nc.dram_tensor(
            "dbg_vaug", [128, lkv // 128, 4, 65], F32, kind="ExternalOutput").ap()
        taps["dbg_rs"] = nc.dram_tensor(
            "dbg_rs", [hd // 128, lq // 512, 2, 512], F32, kind="ExternalOutput").ap()
        taps["dbg_otn"] = nc.dram_tensor(
            "dbg_otn", [128, hd // 128, lq], F32, kind="ExternalOutput").ap()

    ncc = c // 128          # contraction chunks for Q proj (8)
    nctx = ctx // 128       # contraction chunks for K/V proj (6)
    nit = lq // 512         # i tiles (4)
    njt = lkv // 128        # j chunks (16)
    npair = hd // 128       # head pairs (2)
    nct = c // 128          # out column tiles (8)

    with tile.TileContext(nc) as tc:
        with (
            tc.tile_pool(name="big", bufs=1) as big,
            tc.tile_pool(name="wts", bufs=1) as wts,
            tc.tile_pool(name="acts", bufs=1) as acts,
            tc.tile_pool(name="pt", bufs=2) as ptp,
            tc.tile_pool(name="nrm", bufs=4) as nrm,
            tc.tile_pool(name="stg", bufs=2) as stgp,
            tc.tile_pool(name="osb", bufs=3) as osb,
            tc.tile_pool(name="st", bufs=1, space="PSUM") as stp,
            tc.tile_pool(name="ot", bufs=2, space="PSUM") as otp,
        ):
            # ---- persistent activations/weights in SBUF
            qt = acts.tile([128, npair, lq], F32R, tag="qt")      # Q^T pair-stacked
            kt = acts.tile([128, npair, lkv], F32R, tag="kt")     # K^T pair-stacked
            vaug = acts.tile([128, njt, 4, 65], F32R, tag="vaug")  # [V_h | ones] per j-chunk
            otn = acts.tile([128, npair, lq], F32R, tag="otn")    # normalized O^T

            # ---- phase A: Q projection (qt[hd, lq] = wq.T @ x.T)
            x_sb = big.tile([128, ncc, lq], F32R, tag="xy")
            nc.sync.dma_start(
                out=x_sb, in_=xT.rearrange("(cc p) l -> p cc l", p=128).bitcast(F32R))
            wq_sb = wts.tile([128, ncc, hd], F32R, tag="wq")
            nc.sync.dma_start(
                out=wq_sb, in_=wq.rearrange("(cc p) h -> p cc h", p=128).bitcast(F32R))

            for pair in range(npair):
                for it in range(nit):
                    ps = otp.tile([128, 512], F32, tag="ot")
                    for cc in range(ncc):
                        nc.tensor.matmul(
                            ps[:],
                            wq_sb[:, cc, pair * 128:(pair + 1) * 128],
                            x_sb[:, cc, it * 512:(it + 1) * 512],
                            start=(cc == 0), stop=(cc == ncc - 1))
                    nc.vector.tensor_copy(qt[:, pair, it * 512:(it + 1) * 512], ps[:])

            # ---- phase B: K projection and V projection
            y_sb = big.tile([128, nctx, lkv], F32R, tag="xy")
            nc.sync.dma_start(
                out=y_sb, in_=yT.rearrange("(cc p) l -> p cc l", p=128).bitcast(F32R))
            wk_sb = wts.tile([128, nctx, hd], F32R, tag="wk")
            nc.sync.dma_start(
                out=wk_sb, in_=wk.rearrange("(cc p) h -> p cc h", p=128).bitcast(F32R))
            wv_sb = wts.tile([128, nctx, hd], F32R, tag="wv")
            nc.sync.dma_start(
                out=wv_sb, in_=wv.rearrange("(cc p) h -> p cc h", p=128).bitcast(F32R))

            for pair in range(npair):
                for it in range(nit):
                    ps = otp.tile([128, 512], F32, tag="ot")
                    for cc in range(nctx):
                        nc.tensor.matmul(
                            ps[:],
                            wk_sb[:, cc, pair * 128:(pair + 1) * 128],
                            y_sb[:, cc, it * 512:(it + 1) * 512],
                            start=(cc == 0), stop=(cc == nctx - 1))
                    nc.vector.tensor_copy(kt[:, pair, it * 512:(it + 1) * 512], ps[:])

            ones_sb = wts.tile([128, njt, 4], F32, tag="ones")
            nc.vector.memset(ones_sb[:], 1.0)
            nc.vector.tensor_copy(
                vaug[:, :, :, 64:65],
                ones_sb[:].rearrange("p j (h o) -> p j h o", o=1))
            for jt in range(njt):
                ps = otp.tile([128, 256], F32, tag="ot")
                for cc in range(nctx):
                    nc.tensor.matmul(
                        ps[:],
                        y_sb[:, cc, jt * 128:(jt + 1) * 128],
                        wv_sb[:, cc, :],
                        start=(cc == 0), stop=(cc == nctx - 1))
                nc.vector.tensor_copy(
                    vaug[:, jt, :, 0:64],
                    ps[:].rearrange("p (h d) -> p h d", d=64))

            # ---- phase C: attention, per pair / i-tile; flash-style over j
            gmax = min(3, njt)
            groups = [(g0, min(gmax, njt - g0)) for g0 in range(0, njt, gmax)]
            for pair in range(npair):
                ha, hb = 2 * pair, 2 * pair + 1
                for it in range(nit):
                    ot_a = otp.tile([65, 512], F32, tag="ot")
                    ot_b = otp.tile([65, 512], F32, tag="ot")
                    for (g0, glen) in groups:
                        st = stp.tile([128, 2, glen, 512], F32, tag="st")
                        for k in range(glen):
                            jt = g0 + k
                            nc.tensor.matmul(
                                st[:, 0, k, :],
                                kt[0:64, pair, jt * 128:(jt + 1) * 128],
                                qt[0:64, pair, it * 512:(it + 1) * 512],
                                start=True, stop=True)
                            nc.tensor.matmul(
                                st[:, 1, k, :],
                                kt[64:128, pair, jt * 128:(jt + 1) * 128],
                                qt[64:128, pair, it * 512:(it + 1) * 512],
                                start=True, stop=True)
                        pt = ptp.tile([128, 2, gmax, 512], F32R, tag="pt")
                        nc.scalar.activation(
                            pt[:, :, 0:glen, :], st[:],
                            mybir.ActivationFunctionType.Exp, scale=SCALE)
                        for k in range(glen):
                            jt = g0 + k
                            nc.tensor.matmul(
                                ot_a[:], vaug[:, jt, ha, :], pt[:, 0, k, :],
                                start=(jt == 0), stop=(jt == njt - 1))
                            nc.tensor.matmul(
                                ot_b[:], vaug[:, jt, hb, :], pt[:, 1, k, :],
                                start=(jt == 0), stop=(jt == njt - 1))
                    # normalize: O^T[h] / rowsum (row 64 of each ot tile)
                    for h, ot in ((0, ot_a), (1, ot_b)):
                        rs = nrm.tile([65, 512], F32, tag="rs")
                        nc.vector.tensor_copy(rs[64:65, :], ot[64:65, :])
                        if debug_taps:
                            nc.sync.dma_start(out=taps["dbg_rs"][pair, it, h, :],
                                              in_=rs[64:65, :])
                        nc.vector.reciprocal(
                            out=rs[64:65, :], in_=rs[64:65, :])
                        nc.sync.dma_start(out=rsd[pair, it, h, :],
                                          in_=rs[64:65, :])
                        rc = nrm.tile([64, 512], F32, tag="rc")
                        nc.sync.dma_start(
                            out=rc, in_=_bcast_rows(rsd[pair, it, h:h + 1, :], 64))
                        if h == 0:
                            nc.vector.tensor_mul(
                                otn[0:64, pair, it * 512:(it + 1) * 512],
                                ot[0:64, :], rc[:])
                        else:
                            stg = stgp.tile([64, 512], F32R, tag="stg")
                            nc.vector.tensor_mul(stg[:], ot[0:64, :], rc[:])
                            nc.sync.dma_start(
                                out=otn[64:128, pair, it * 512:(it + 1) * 512],
                                in_=stg[:])

            if debug_taps:
                nc.sync.dma_start(out=taps["dbg_qt"], in_=qt[:].bitcast(F32))
                nc.sync.dma_start(out=taps["dbg_kt"], in_=kt[:].bitcast(F32))
                nc.sync.dma_start(out=taps["dbg_vaug"], in_=vaug[:].bitcast(F32))
                nc.sync.dma_start(out=taps["dbg_otn"], in_=otn[:].bitcast(F32))

            # ---- phase D: output projection outT[ct, it] += wp.T @ otn
            wp_sb = wts.tile([128, npair, c], F32R, tag="wp")
            nc.sync.dma_start(
                out=wp_sb, in_=wp.rearrange("(r p) o -> p r o", p=128).bitcast(F32R))
            for ct in range(nct):
                for it in range(nit):
                    ps = otp.tile([128, 512], F32, tag="ot")
                    for pair in range(npair):
                        nc.tensor.matmul(
                            ps[:],
                            wp_sb[:, pair, ct * 128:(ct + 1) * 128],
                            otn[:, pair, it * 512:(it + 1) * 512],
                            start=(pair == 0), stop=(pair == npair - 1))
                    o_sb = osb.tile([128, 512], F32, tag="osb")
                    nc.vector.tensor_copy(o_sb[:], ps[:])
                    nc.sync.dma_start(
                        out=outT[ct * 128:(ct + 1) * 128, it * 512:(it + 1) * 512],
                        in_=o_sb[:])

    nc.compile()
    return nc


_NC_CACHE = {}


def _get_nc():
    if "nc" not in _NC_CACHE:
        _NC_CACHE["nc"] = build_kernel()
    return _NC_CACHE["nc"]


def make_in_maps(x, y, Wq, Wkv, Wproj):
    """Host-side sharding: core = b * 4 + hg (hg = 4-head group)."""
    x = np.asarray(x, dtype=np.float32)
    y = np.asarray(y, dtype=np.float32)
    Wq = np.asarray(Wq, dtype=np.float32)
    Wkv = np.asarray(Wkv, dtype=np.float32).reshape(CTX, 2, H, DK)
    Wproj = np.asarray(Wproj, dtype=np.float32)

    in_maps = []
    for core in range(8):
        b, hg = core // 4, core % 4
        hs = slice(4 * hg, 4 * hg + 4)
        in_maps.append({
            "xT": np.ascontiguousarray(x[b].T),
            "yT": np.ascontiguousarray(y[b].T),
            "wq": np.ascontiguousarray(Wq[:, 4 * hg * DK:(4 * hg + 4) * DK]),
            "wk": np.ascontiguousarray(Wkv[:, 0, hs, :].reshape(CTX, 4 * DK)),
            "wv": np.ascontiguousarray(Wkv[:, 1, hs, :].reshape(CTX, 4 * DK)),
            "wp": np.ascontiguousarray(Wproj[4 * hg * DK:(4 * hg + 4) * DK, :]),
        })
    return in_maps


def kernel(x, y, Wq, Wkv, Wproj, bproj):
    nc = _get_nc()
    in_maps = make_in_maps(x, y, Wq, Wkv, Wproj)
    res = run_bass_kernel_spmd(nc, in_maps, core_ids=list(range(8)))
    bproj = np.asarray(bproj, dtype=np.float32)
    out = np.empty((B, LQ, C), dtype=np.float32)
    for b in range(B):
        acc = res.results[4 * b]["outT"].astype(np.float32).copy()
        for hg in range(1, 4):
            acc += res.results[4 * b + hg]["outT"]
        out[b] = acc.T + bproj
    return out



# revision 2
# speedup vs baseline: 1.4786x; 1.4786x over previous
"""CrossAttentionBlock kernel for 8 Trainium2 NeuronCores.

Reference computation (per batch b):
    q = x @ Wq;  k,v = y @ Wkv;  per head: softmax(q k^T / sqrt(dk)) v;
    out = concat_heads @ Wproj + bproj

Sharding: 8 cores = 2 batches x 4 head-groups (4 heads each). Each core
computes the partial output contribution of its 4 heads for its batch;
the host sums the 4 partials per batch and adds the bias.

Design notes (cost-model driven):
  - All matmul operands fp16 (1 cycle/row on PE at any size), PSUM fp32.
  - Attention AV is computed in [i, d] orientation (lhsT = P tile), which
    costs 65 rows per j-chunk instead of 512 -> half the PE rows of the
    S^T orientation, and makes the softmax normalization a per-partition
    scalar multiply (no DRAM broadcast bounce).
  - Rowsums come free as a 65th "ones" column of V.
  - O^T for the output projection is produced by DMA XBAR transposes
    (16-bit, [128,128] tiles) - no PE/DVE cost.
  - The PE clock ramps (0.65 -> 1.2 -> 2.4 GHz) and resets on idle, so the
    program is scheduled to keep PE continuously busy: projection work for
    pair 1 / output projection is interleaved as "filler" into the
    attention pipeline of the current pair, and exp runs on Act in 3-chunk
    groups double-buffered in PSUM so neither engine waits.

Per-core layout (host prepares, fp16):
    xT  [1024, 2048]  x[b].T            (contraction dim on partitions)
    yT  [768, 2048]   y[b].T
    wq  [1024, 256]   Wq columns of this head group
    wk  [768, 256]    K-half of Wkv for this head group
    wv  [768, 256]    V-half of Wkv for this head group
    wp  [256, 1024]   Wproj rows of this head group
Output:
    outT [1024, 2048] fp32 partial (x @ .. @ Wproj).T for this head group
"""

import numpy as np

import concourse.bass as bass
import concourse.tile as tile
from concourse import bacc, mybir
from concourse.bass_utils import run_bass_kernel_spmd

B, LQ, LKV = 2, 2048, 2048
C, CTX, H, DK = 1024, 768, 16, 64
SCALE = DK ** (-0.5)

F32 = mybir.dt.float32
F16 = mybir.dt.float16

NCC = C // 128       # 8   contraction chunks for Q proj
NCTX = CTX // 128    # 6   contraction chunks for K/V proj
NIT = LQ // 512      # 4   i blocks
NJT = LKV // 128     # 16  j chunks
GROUPS = [(0, 3), (3, 3), (6, 3), (9, 3), (12, 3), (15, 1)]


def build_kernel():
    nc = bacc.Bacc("TRN2", target_bir_lowering=False, debug=False)

    xT = nc.dram_tensor("xT", [C, LQ], F16, kind="ExternalInput").ap()
    yT = nc.dram_tensor("yT", [CTX, LKV], F16, kind="ExternalInput").ap()
    wq = nc.dram_tensor("wq", [C, 256], F16, kind="ExternalInput").ap()
    wk = nc.dram_tensor("wk", [CTX, 256], F16, kind="ExternalInput").ap()
    wv = nc.dram_tensor("wv", [CTX, 256], F16, kind="ExternalInput").ap()
    wp = nc.dram_tensor("wp", [256, C], F16, kind="ExternalInput").ap()
    outT = nc.dram_tensor("outT", [C, LQ], F32, kind="ExternalOutput").ap()

    xTr = xT.rearrange("(cc p) l -> p cc l", p=128)
    yTr = yT.rearrange("(cc p) l -> p cc l", p=128)
    outTr = outT.rearrange("(ct p) l -> p ct l", p=128)

    with tile.TileContext(nc) as tc:
        with (
            tc.tile_pool(name="prs", bufs=1) as prs,      # persistent SBUF
            tc.tile_pool(name="pt", bufs=4) as ptp,       # exp outputs
            tc.tile_pool(name="nrm", bufs=4) as nrm,      # 1/rowsum
            tc.tile_pool(name="stg", bufs=6) as stg,      # normalized O staging
            tc.tile_pool(name="ob", bufs=2) as obp,       # output staging
            tc.tile_pool(name="st", bufs=2, space="PSUM") as stp,   # scores + proj
            tc.tile_pool(name="ot", bufs=2, space="PSUM") as otp,   # AV accum
        ):
            # ---- persistent SBUF tensors
            x_sb = prs.tile([128, NCC, LQ], F16, tag="x")
            y_sb = prs.tile([128, NCTX, LKV], F16, tag="y")
            wq_sb = prs.tile([128, NCC, 256], F16, tag="wq")
            wk_sb = prs.tile([128, NCTX, 256], F16, tag="wk")
            wv_sb = prs.tile([128, NCTX, 256], F16, tag="wv")
            wp_sb = prs.tile([128, 2, C], F16, tag="wp")
            qt = prs.tile([128, 2, LQ], F16, tag="qt")      # [d(2h), pair, i]
            kt = prs.tile([128, 2, LKV], F16, tag="kt")     # [d(2h), pair, j]
            vaug = prs.tile([128, NJT, 4, 65], F16, tag="va")  # [j, jt, h, d|1]
            otn = prs.tile([128, 2, LQ], F16, tag="otn")    # [hd(2h), pair, i]
            ones = prs.tile([128, NJT, 4], F16, tag="ones")
            scr = prs.tile([1, 2], F16, tag="scr")          # act-table warm

            # ---- DMAs (SP queue, ordered to match consumption order)
            # Act Exp table preload (overlaps the first DMA waits)
            nc.vector.memset(scr[:], 0.0)
            nc.scalar.activation(scr[0:1, 0:1], scr[0:1, 1:2],
                                 mybir.ActivationFunctionType.Exp)

            nc.sync.dma_start(out=x_sb[:, :, 0:512], in_=xTr[:, :, 0:512])
            nc.sync.dma_start(out=wq_sb, in_=wq.rearrange("(cc p) h -> p cc h", p=128))
            nc.sync.dma_start(out=y_sb[:, :, 0:512], in_=yTr[:, :, 0:512])
            nc.sync.dma_start(out=wk_sb, in_=wk.rearrange("(cc p) h -> p cc h", p=128))
            nc.sync.dma_start(out=wv_sb, in_=wv.rearrange("(cc p) h -> p cc h", p=128))
            for blk in range(1, 4):
                s = slice(blk * 512, (blk + 1) * 512)
                nc.sync.dma_start(out=x_sb[:, :, s], in_=xTr[:, :, s])
                nc.sync.dma_start(out=y_sb[:, :, s], in_=yTr[:, :, s])
            nc.sync.dma_start(out=wp_sb, in_=wp.rearrange("(r p) o -> p r o", p=128))

            nc.vector.memset(ones[:], 1.0)
            nc.vector.tensor_copy(
                vaug[:, :, :, 64:65],
                ones[:].rearrange("p j (h o) -> p j h o", o=1))

            # ---- unit emitters (each is an independent chunk of PE work)
            def q_unit(pair, it):
                ps = stp.tile([128, 512], F32, tag="st")
                for cc in range(NCC):
                    nc.tensor.matmul(
                        ps[:], wq_sb[:, cc, pair * 128:(pair + 1) * 128],
                        x_sb[:, cc, it * 512:(it + 1) * 512],
                        start=(cc == 0), stop=(cc == NCC - 1))
                nc.vector.tensor_copy(qt[:, pair, it * 512:(it + 1) * 512], ps[:])

            def k_unit(pair, it):
                ps = stp.tile([128, 512], F32, tag="st")
                for cc in range(NCTX):
                    nc.tensor.matmul(
                        ps[:], wk_sb[:, cc, pair * 128:(pair + 1) * 128],
                        y_sb[:, cc, it * 512:(it + 1) * 512],
                        start=(cc == 0), stop=(cc == NCTX - 1))
                nc.vector.tensor_copy(kt[:, pair, it * 512:(it + 1) * 512], ps[:])

            def v_unit(pair, jt):
                # V proj for the 2 heads of `pair`, j-chunk jt
                ps = stp.tile([128, 128], F32, tag="st")
                for cc in range(NCTX):
                    nc.tensor.matmul(
                        ps[:], y_sb[:, cc, jt * 128:(jt + 1) * 128],
                        wv_sb[:, cc, pair * 128:(pair + 1) * 128],
                        start=(cc == 0), stop=(cc == NCTX - 1))
                nc.vector.tensor_copy(
                    vaug[:, jt, 2 * pair:2 * pair + 2, 0:64],
                    ps[:].rearrange("p (h d) -> p h d", d=64))

            def d_unit(it, ct, ob):
                ps = stp.tile([128, 512], F32, tag="st")
                for pair in range(2):
                    nc.tensor.matmul(
                        ps[:], wp_sb[:, pair, ct * 128:(ct + 1) * 128],
                        otn[:, pair, it * 512:(it + 1) * 512],
                        start=(pair == 0), stop=(pair == 1))
                nc.vector.tensor_copy(ob[:, ct, :], ps[:])

            # ---- filler queue: keeps PE busy while Act runs exp
            filler = []

            def fill(n=1):
                for _ in range(n):
                    if filler:
                        filler.pop(0)()

            # ---- attention for one (pair, it, h): scores -> exp -> AV
            def attn_head(pair, it, h, ot_h):
                d0 = 64 * h
                for gi, (g0, glen) in enumerate(GROUPS):
                    st = stp.tile([128, glen, 512], F32, tag="st")
                    for k in range(glen):
                        jt = g0 + k
                        nc.tensor.matmul(
                            st[:, k, :],
                            kt[d0:d0 + 64, pair, jt * 128:(jt + 1) * 128],
                            qt[d0:d0 + 64, pair, it * 512:(it + 1) * 512],
                            start=True, stop=True)
                    pt = ptp.tile([128, glen, 512], F16, tag="pt")
                    nc.scalar.activation(
                        pt[:], st[:], mybir.ActivationFunctionType.Exp,
                        scale=SCALE)
                    if gi % 2 == 0:
                        fill()
                    for k in range(glen):
                        jt = g0 + k
                        for isub in range(4):
                            nc.tensor.matmul(
                                ot_h[:, isub, 0:65],
                                pt[:, k, isub * 128:(isub + 1) * 128],
                                vaug[:, jt, 2 * pair + h, :],
                                start=(jt == 0), stop=(jt == NJT - 1))

            # ---- normalization + transpose for one (pair, it)
            def norm_transpose(pair, it, ot_a, ot_b):
                rs = []
                for h, ot_h in ((0, ot_a), (1, ot_b)):
                    rsinv = nrm.tile([128, 4], F32, tag="rs")
                    nc.vector.reciprocal(rsinv[:], ot_h[:, :, 64:65])
                    rs.append(rsinv)
                for isub in range(4):
                    onrm = stg.tile([128, 128], F16, tag="onrm")
                    for h, ot_h in ((0, ot_a), (1, ot_b)):
                        nc.vector.tensor_scalar_mul(
                            onrm[:, 64 * h:64 * h + 64],
                            ot_h[:, isub, 0:64],
                            rs[h][:, isub:isub + 1])
                    base = it * 512 + isub * 128
                    nc.sync.dma_start(
                        out=otn[:, pair, base:base + 128], in_=onrm[:],
                        transpose=True)

            # ---- prefix: minimal pair-0 prerequisites (PE ramps up here)
            q_unit(0, 0)
            k_unit(0, 0)
            for jt in range(4):
                v_unit(0, jt)

            # remaining prep work becomes filler inside C0
            for it in range(1, 4):
                filler.append(lambda it=it: q_unit(0, it))
            filler.append(lambda: k_unit(0, 1))
            for jt in range(4, 8):
                filler.append(lambda jt=jt: v_unit(0, jt))
            filler.append(lambda: k_unit(0, 2))
            for jt in range(8, 12):
                filler.append(lambda jt=jt: v_unit(0, jt))
            filler.append(lambda: k_unit(0, 3))
            for jt in range(12, 16):
                filler.append(lambda jt=jt: v_unit(0, jt))
            # pair-1 prep: also filler, consumed across C0 (K/V needed by C1
            # start; Q1(it) just-in-time during C1)
            for it in range(4):
                filler.append(lambda it=it: k_unit(1, it))
            for jt in range(16):
                filler.append(lambda jt=jt: v_unit(1, jt))

            # ---- C0: attention pair 0 (filler interleaved)
            for it in range(4):
                ot_a = otp.tile([128, 4, 128], F32, tag="ot")
                ot_b = otp.tile([128, 4, 128], F32, tag="ot")
                attn_head(0, it, 0, ot_a)
                attn_head(0, it, 1, ot_b)
                norm_transpose(0, it, ot_a, ot_b)
            fill(len(filler))  # flush any leftover pair-0/1 prep

            # Q1 units ride inside C1 just-in-time
            for it in range(4):
                filler.append(lambda it=it: q_unit(1, it))

            # ---- C1: attention pair 1, with output projection interleaved
            obs = [None] * 4
            for it in range(4):
                ot_a = otp.tile([128, 4, 128], F32, tag="ot")
                ot_b = otp.tile([128, 4, 128], F32, tag="ot")
                if it > 0:
                    ob = obp.tile([128, 8, 512], F32, tag="ob")
                    obs[it - 1] = ob
                    for ct in range(8):
                        filler.append(
                            lambda it=it, ct=ct, ob=ob: d_unit(it - 1, ct, ob))
                attn_head(1, it, 0, ot_a)
                attn_head(1, it, 1, ot_b)
                norm_transpose(1, it, ot_a, ot_b)
                if it > 0:
                    fill(len(filler))  # ensure D(it-1) done before store
                    nc.sync.dma_start(
                        out=outTr[:, :, (it - 1) * 512:it * 512],
                        in_=obs[it - 1])

            # ---- tail: D(3)
            ob = obp.tile([128, 8, 512], F32, tag="ob")
            for ct in range(8):
                d_unit(3, ct, ob)
            nc.sync.dma_start(out=outTr[:, :, 3 * 512:4 * 512], in_=ob)

    nc.compile()
    return nc


_NC_CACHE = {}


def _get_nc():
    if "nc" not in _NC_CACHE:
        _NC_CACHE["nc"] = build_kernel()
    return _NC_CACHE["nc"]


def make_in_maps(x, y, Wq, Wkv, Wproj):
    """Host-side sharding: core = b * 4 + hg (hg = 4-head group)."""
    x = np.asarray(x, dtype=np.float32)
    y = np.asarray(y, dtype=np.float32)
    Wq = np.asarray(Wq, dtype=np.float32)
    Wkv = np.asarray(Wkv, dtype=np.float32).reshape(CTX, 2, H, DK)
    Wproj = np.asarray(Wproj, dtype=np.float32)

    in_maps = []
    for core in range(8):
        b, hg = core // 4, core % 4
        hs = slice(4 * hg, 4 * hg + 4)
        in_maps.append({
            "xT": np.ascontiguousarray(x[b].T).astype(np.float16),
            "yT": np.ascontiguousarray(y[b].T).astype(np.float16),
            "wq": np.ascontiguousarray(
                Wq[:, 4 * hg * DK:(4 * hg + 4) * DK]).astype(np.float16),
            "wk": np.ascontiguousarray(
                Wkv[:, 0, hs, :].reshape(CTX, 4 * DK)).astype(np.float16),
            "wv": np.ascontiguousarray(
                Wkv[:, 1, hs, :].reshape(CTX, 4 * DK)).astype(np.float16),
            "wp": np.ascontiguousarray(
                Wproj[4 * hg * DK:(4 * hg + 4) * DK, :]).astype(np.float16),
        })
    return in_maps


def kernel(x, y, Wq, Wkv, Wproj, bproj):
    nc = _get_nc()
    in_maps = make_in_maps(x, y, Wq, Wkv, Wproj)
    res = run_bass_kernel_spmd(nc, in_maps, core_ids=list(range(8)))
    bproj = np.asarray(bproj, dtype=np.float32)
    out = np.empty((B, LQ, C), dtype=np.float32)
    for b in range(B):
        acc = res.results[4 * b]["outT"].astype(np.float32).copy()
        for hg in range(1, 4):
            acc += res.results[4 * b + hg]["outT"]
        out[b] = acc.T + bproj
    return out


# revision 6
# speedup vs baseline: 1.9070x; 1.2897x over previous
"""CrossAttentionBlock kernel for 8 Trainium2 NeuronCores.

Reference computation (per batch b):
    q = x @ Wq;  k,v = y @ Wkv;  per head: softmax(q k^T / sqrt(dk)) v;
    out = concat_heads @ Wproj + bproj

Sharding: 8 cores = 2 batches x 4 head-groups (4 heads each). Each core
computes the partial output contribution of its 4 heads for its batch;
the host sums the 4 partials per batch and adds the bias.

Design notes (cost-model driven):
  - All matmul operands fp16 (1 cycle/row on PE at any size), PSUM fp32.
  - Attention AV is computed in [i, d] orientation (lhsT = P tile), which
    costs 65 rows per j-chunk instead of 512 -> half the PE rows of the
    S^T orientation, and makes the softmax normalization a per-partition
    scalar multiply (no DRAM broadcast bounce). Rowsums come free as a
    65th "ones" column of V.
  - O^T for the output projection is produced by DMA XBAR transposes
    (16-bit, [128,128] tiles) - no PE/DVE cost.
  - The PE clock ramps (0.65 -> 1.2 -> 2.4 GHz) and resets on idle, so the
    schedule keeps PE continuously busy: a flat software pipeline over
    (pair, i-block, head, j-group) runs scores 2 slots ahead of the AV
    consumer of the exp output, and all projection work (Q/K/V of the
    other pair, output projection) is interleaved as filler units.
"""

import numpy as np

import concourse.bass as bass
import concourse.tile as tile
from concourse import bacc, mybir
from concourse.bass_utils import run_bass_kernel_spmd

B, LQ, LKV = 2, 2048, 2048
C, CTX, H, DK = 1024, 768, 16, 64
SCALE = DK ** (-0.5)

F32 = mybir.dt.float32
F16 = mybir.dt.float16

NCC = C // 128       # 8   contraction chunks for Q proj
NCTX = CTX // 128    # 6   contraction chunks for K/V proj
NIT = LQ // 512      # 4   i blocks
NJT = LKV // 128     # 16  j chunks
GROUPS = [(g0, 2) for g0 in range(0, NJT, 2)]   # 8 groups of 2 j-chunks


def build_kernel():
    nc = bacc.Bacc("TRN2", target_bir_lowering=False, debug=False)

    xT = nc.dram_tensor("xT", [C, LQ], F16, kind="ExternalInput").ap()
    yT = nc.dram_tensor("yT", [CTX, LKV], F16, kind="ExternalInput").ap()
    wq = nc.dram_tensor("wq", [C, 256], F16, kind="ExternalInput").ap()
    wk = nc.dram_tensor("wk", [CTX, 256], F16, kind="ExternalInput").ap()
    wv = nc.dram_tensor("wv", [CTX, 256], F16, kind="ExternalInput").ap()
    wp = nc.dram_tensor("wp", [256, C], F16, kind="ExternalInput").ap()
    outT = nc.dram_tensor("outT", [C, LQ], F32, kind="ExternalOutput").ap()

    xTr = xT.rearrange("(cc p) l -> p cc l", p=128)
    yTr = yT.rearrange("(cc p) l -> p cc l", p=128)
    outTr = outT.rearrange("(ct p) l -> p ct l", p=128)

    with tile.TileContext(nc) as tc:
        with (
            tc.tile_pool(name="prs", bufs=1) as prs,      # persistent SBUF
            tc.tile_pool(name="pt", bufs=4) as ptp,       # exp outputs
            tc.tile_pool(name="nrm", bufs=4) as nrm,      # 1/rowsum
            tc.tile_pool(name="stg", bufs=6) as stg,      # normalized O staging
            tc.tile_pool(name="ob", bufs=2) as obp,       # output staging
            tc.tile_pool(name="st", bufs=2, space="PSUM") as stp,   # scores/proj
            tc.tile_pool(name="ot", bufs=2, space="PSUM") as otp,   # AV accum
        ):
            # ---- persistent SBUF tensors
            x_sb = prs.tile([128, NCC, LQ], F16, tag="x")
            y_sb = prs.tile([128, NCTX, LKV], F16, tag="y")
            wq_sb = prs.tile([128, NCC, 256], F16, tag="wq")
            wk_sb = prs.tile([128, NCTX, 256], F16, tag="wk")
            wv_sb = prs.tile([128, NCTX, 256], F16, tag="wv")
            wp_sb = prs.tile([128, 2, C], F16, tag="wp")
            qt = prs.tile([128, 2, LQ], F16, tag="qt")      # [d(2h), pair, i]
            kt = prs.tile([128, 2, LKV], F16, tag="kt")     # [d(2h), pair, j]
            vaug = prs.tile([128, NJT, 4, 65], F16, tag="va")  # [j, jt, h, d|1]
            otn = prs.tile([128, 2, LQ], F16, tag="otn")    # [hd(2h), pair, i]
            ones = prs.tile([128, NJT, 4], F16, tag="ones")
            scr = prs.tile([1, 2], F16, tag="scr")          # act-table warm

            # ---- DMAs (SP queue, ordered to match consumption order)
            nc.vector.memset(scr[:], 0.0)
            nc.scalar.activation(scr[0:1, 0:1], scr[0:1, 1:2],
                                 mybir.ActivationFunctionType.Exp)

            nc.sync.dma_start(out=wq_sb, in_=wq.rearrange("(cc p) h -> p cc h", p=128))
            nc.sync.dma_start(out=x_sb[:, :, 0:512], in_=xTr[:, :, 0:512])
            nc.sync.dma_start(out=wk_sb, in_=wk.rearrange("(cc p) h -> p cc h", p=128))
            nc.sync.dma_start(out=y_sb[:, :, 0:512], in_=yTr[:, :, 0:512])
            nc.sync.dma_start(out=wv_sb, in_=wv.rearrange("(cc p) h -> p cc h", p=128))
            for blk in range(1, 4):
                s = slice(blk * 512, (blk + 1) * 512)
                nc.sync.dma_start(out=x_sb[:, :, s], in_=xTr[:, :, s])
                nc.sync.dma_start(out=y_sb[:, :, s], in_=yTr[:, :, s])
            nc.sync.dma_start(out=wp_sb, in_=wp.rearrange("(r p) o -> p r o", p=128))

            nc.vector.memset(ones[:], 1.0)
            nc.vector.tensor_copy(
                vaug[:, :, :, 64:65],
                ones[:].rearrange("p j (h o) -> p j h o", o=1))

            # ---- unit emitters (independent chunks of PE work)
            def q_unit(pair, it):
                ps = stp.tile([128, 512], F32, tag="ps")
                for cc in range(NCC):
                    nc.tensor.matmul(
                        ps[:], wq_sb[:, cc, pair * 128:(pair + 1) * 128],
                        x_sb[:, cc, it * 512:(it + 1) * 512],
                        start=(cc == 0), stop=(cc == NCC - 1))
                nc.vector.tensor_copy(qt[:, pair, it * 512:(it + 1) * 512], ps[:])

            def k_unit(pair, it):
                ps = stp.tile([128, 512], F32, tag="ps")
                for cc in range(NCTX):
                    nc.tensor.matmul(
                        ps[:], wk_sb[:, cc, pair * 128:(pair + 1) * 128],
                        y_sb[:, cc, it * 512:(it + 1) * 512],
                        start=(cc == 0), stop=(cc == NCTX - 1))
                nc.vector.tensor_copy(kt[:, pair, it * 512:(it + 1) * 512], ps[:])

            def v_unit(pair, jt):
                # V proj for the 2 heads of `pair`, j-chunk jt
                ps = stp.tile([128, 128], F32, tag="ps")
                for cc in range(NCTX):
                    nc.tensor.matmul(
                        ps[:], y_sb[:, cc, jt * 128:(jt + 1) * 128],
                        wv_sb[:, cc, pair * 128:(pair + 1) * 128],
                        start=(cc == 0), stop=(cc == NCTX - 1))
                nc.vector.tensor_copy(
                    vaug[:, jt, 2 * pair:2 * pair + 2, 0:64],
                    ps[:].rearrange("p (h d) -> p h d", d=64))

            def d_unit(it, ct, ob, nsub=1):
                # output projection for column-tile ct of i-block it.
                # nsub>1 splits the moving dim so each matmul only depends on
                # a single XBAR transpose (used for the tail i-block).
                ps = stp.tile([128, 512], F32, tag="ps")
                w = 512 // nsub
                for s in range(nsub):
                    for pair in range(2):
                        nc.tensor.matmul(
                            ps[:, s * w:(s + 1) * w],
                            wp_sb[:, pair, ct * 128:(ct + 1) * 128],
                            otn[:, pair, it * 512 + s * w:it * 512 + (s + 1) * w],
                            start=(pair == 0), stop=(pair == 1))
                nc.vector.tensor_copy(ob[:, ct, :], ps[:])

            # ---- filler scheduling: units are popped one per pipeline slot;
            # pushes can be delayed by slots (XBAR transpose latency).
            filler = []
            pending = []  # [slots_left, [closures]]

            def push(units, delay=0):
                if delay:
                    pending.append([delay, list(units)])
                else:
                    filler.extend(units)

            def fill():
                for p in pending:
                    p[0] -= 1
                while pending and pending[0][0] <= 0:
                    filler.extend(pending.pop(0)[1])
                if filler:
                    filler.pop(0)()

            # ---- attention pipeline pieces
            def scores_exp(pair, it, h, g0, glen):
                d0 = 64 * h
                st = stp.tile([128, glen, 512], F32, tag="st")
                for k in range(glen):
                    jt = g0 + k
                    nc.tensor.matmul(
                        st[:, k, :],
                        kt[d0:d0 + 64, pair, jt * 128:(jt + 1) * 128],
                        qt[d0:d0 + 64, pair, it * 512:(it + 1) * 512],
                        start=True, stop=True)
                pt = ptp.tile([128, glen, 512], F16, tag="pt")
                nc.scalar.activation(
                    pt[:], st[:], mybir.ActivationFunctionType.Exp, scale=SCALE)
                return pt

            ots = {}   # (pair, it) -> (ot_a, ot_b)
            obs = {}   # it -> output staging tile

            def av(pair, it, h, g0, glen, pt):
                if h == 0 and g0 == 0:
                    ot_a = otp.tile([128, 4, 128], F32, tag="ot", name="ot_a")
                    ot_b = otp.tile([128, 4, 128], F32, tag="ot", name="ot_b")
                    ots[(pair, it)] = (ot_a, ot_b)
                ot_h = ots[(pair, it)][h]
                for k in range(glen):
                    jt = g0 + k
                    for isub in range(4):
                        nc.tensor.matmul(
                            ot_h[:, isub, 0:65],
                            pt[:, k, isub * 128:(isub + 1) * 128],
                            vaug[:, jt, 2 * pair + h, :],
                            start=(jt == 0), stop=(jt == NJT - 1))

            def norm_transpose(pair, it):
                ot_a, ot_b = ots.pop((pair, it))
                rs = []
                for h, ot_h in ((0, ot_a), (1, ot_b)):
                    rsinv = nrm.tile([128, 4], F32, tag="rs")
                    nc.vector.reciprocal(rsinv[:], ot_h[:, :, 64:65])
                    rs.append(rsinv)
                for isub in range(4):
                    onrm = stg.tile([128, 128], F16, tag="onrm")
                    for h, ot_h in ((0, ot_a), (1, ot_b)):
                        nc.vector.tensor_scalar_mul(
                            onrm[:, 64 * h:64 * h + 64],
                            ot_h[:, isub, 0:64],
                            rs[h][:, isub:isub + 1])
                    base = it * 512 + isub * 128
                    nc.sync.dma_start(
                        out=otn[:, pair, base:base + 128], in_=onrm[:],
                        transpose=True)

            def on_it_boundary(pair, it):
                """Called right after the final AV of (pair, it) was emitted."""
                norm_transpose(pair, it)
                if pair == 0 and it == 2:
                    push([lambda: q_unit(1, 0)])
                if pair == 0 and it == 3:
                    push([lambda: q_unit(1, 1)])
                if pair == 1:
                    if it + 2 < NIT:
                        push([lambda: q_unit(1, it + 2)])
                    ob = obp.tile([128, 8, 512], F32, tag="ob")
                    obs[it] = ob
                    # D(it) waits 3 slots for the XBAR transposes to land
                    push([lambda ct=ct: d_unit(it, ct, ob) for ct in range(8)],
                         delay=3)
                    if it > 0:
                        # D(it-1) was fully emitted during this i-block
                        nc.sync.dma_start(
                            out=outTr[:, :, (it - 1) * 512:it * 512],
                            in_=obs[it - 1])

            # ---- prefix: minimal pair-0 prerequisites (PE ramps up here)
            q_unit(0, 0)
            k_unit(0, 0)
            for jt in range(4):
                v_unit(0, jt)

            # remaining prep becomes filler, ordered by first consumption
            push([lambda it=it: q_unit(0, it) for it in range(1, 4)])
            push([lambda: k_unit(0, 1)])
            push([lambda jt=jt: v_unit(0, jt) for jt in range(4, 8)])
            push([lambda: k_unit(0, 2)])
            push([lambda jt=jt: v_unit(0, jt) for jt in range(8, 12)])
            push([lambda: k_unit(0, 3)])
            push([lambda jt=jt: v_unit(0, jt) for jt in range(12, 16)])
            push([lambda it=it: k_unit(1, it) for it in range(4)])
            push([lambda jt=jt: v_unit(1, jt) for jt in range(16)])

            # ---- main pipeline: scores/exp run 2 slots ahead of AV
            items = [(pair, it, h, g0, glen)
                     for pair in range(2) for it in range(NIT) for h in range(2)
                     for (g0, glen) in GROUPS]
            inflight = []   # [(item, pt), ...] exp issued, AV not yet emitted

            def retire(slot_item):
                pair, it, h, g0, glen, pt = slot_item
                av(pair, it, h, g0, glen, pt)
                if h == 1 and g0 + glen == NJT:
                    on_it_boundary(pair, it)

            for item in items:
                pair, it, h, g0, glen = item
                pt = scores_exp(pair, it, h, g0, glen)
                inflight.append((pair, it, h, g0, glen, pt))
                if len(inflight) > 2:
                    retire(inflight.pop(0))
                fill()
            while inflight:
                retire(inflight.pop(0))
                fill()
            for _ in range(8):
                fill()

            # ---- tail: D(3) split per-isub (each matmul only waits on one
            # XBAR transpose), then small stores
            nc.sync.dma_start(out=outTr[:, :, 2 * 512:3 * 512], in_=obs[2])
            ob3 = obp.tile([128, 8, 512], F32, tag="ob")
            for ct in range(8):
                d_unit(3, ct, ob3, nsub=4)
                if ct % 2 == 1:
                    nc.sync.dma_start(
                        out=outTr[:, ct - 1:ct + 1, 3 * 512:4 * 512],
                        in_=ob3[:, ct - 1:ct + 1, :])

    nc.compile()
    return nc


_NC_CACHE = {}


def _get_nc():
    if "nc" not in _NC_CACHE:
        _NC_CACHE["nc"] = build_kernel()
    return _NC_CACHE["nc"]


def make_in_maps(x, y, Wq, Wkv, Wproj):
    """Host-side sharding: core = b * 4 + hg (hg = 4-head group)."""
    x = np.asarray(x, dtype=np.float32)
    y = np.asarray(y, dtype=np.float32)
    Wq = np.asarray(Wq, dtype=np.float32)
    Wkv = np.asarray(Wkv, dtype=np.float32).reshape(CTX, 2, H, DK)
    Wproj = np.asarray(Wproj, dtype=np.float32)

    in_maps = []
    for core in range(8):
        b, hg = core // 4, core % 4
        hs = slice(4 * hg, 4 * hg + 4)
        in_maps.append({
            "xT": np.ascontiguousarray(x[b].T).astype(np.float16),
            "yT": np.ascontiguousarray(y[b].T).astype(np.float16),
            "wq": np.ascontiguousarray(
                Wq[:, 4 * hg * DK:(4 * hg + 4) * DK]).astype(np.float16),
            "wk": np.ascontiguousarray(
                Wkv[:, 0, hs, :].reshape(CTX, 4 * DK)).astype(np.float16),
            "wv": np.ascontiguousarray(
                Wkv[:, 1, hs, :].reshape(CTX, 4 * DK)).astype(np.float16),
            "wp": np.ascontiguousarray(
                Wproj[4 * hg * DK:(4 * hg + 4) * DK, :]).astype(np.float16),
        })
    return in_maps


def kernel(x, y, Wq, Wkv, Wproj, bproj):
    nc = _get_nc()
    in_maps = make_in_maps(x, y, Wq, Wkv, Wproj)
    res = run_bass_kernel_spmd(nc, in_maps, core_ids=list(range(8)))
    bproj = np.asarray(bproj, dtype=np.float32)
    out = np.empty((B, LQ, C), dtype=np.float32)
    for b in range(B):
        acc = res.results[4 * b]["outT"].astype(np.float32).copy()
        for hg in range(1, 4):
            acc += res.results[4 * b + hg]["outT"]
        out[b] = acc.T + bproj
    return out


# revision 7
# speedup vs baseline: 2.1313x; 1.1176x over previous
"""CrossAttentionBlock kernel for 8 Trainium2 NeuronCores.

Reference computation (per batch b):
    q = x @ Wq;  k,v = y @ Wkv;  per head: softmax(q k^T / sqrt(dk)) v;
    out = concat_heads @ Wproj + bproj

Sharding: 8 cores = 2 batches x 4 head-groups (4 heads each). Each core
computes the partial output contribution of its 4 heads for its batch;
the host sums the 4 partials per batch and adds the bias.

Design notes (cost-model driven):
  - All matmul operands fp16 (1 cycle/row on PE at any size), PSUM fp32.
  - Attention AV is computed in [i, d] orientation (lhsT = P tile), which
    costs 65 rows per j-chunk instead of 512 -> half the PE rows of the
    S^T orientation, and makes the softmax normalization a per-partition
    scalar multiply (no DRAM broadcast bounce). Rowsums come free as a
    65th "ones" column of V.
  - O^T for the output projection comes from PE transposes (fp16, 128
    rows per [128,64] block) collected in a PSUM fp16 tile, then one DVE
    copy per i-block.
  - The PE clock ramps (0.65 -> 1.2 -> 2.4 GHz) and resets on idle, so the
    schedule keeps PE continuously busy: a flat software pipeline over
    (pair, i-block, head, j-group) runs scores 2 slots ahead of the AV
    consumer of the exp output; projection work (Q/K/V of the other pair,
    transposes, output projection) is interleaved as filler units paced
    by PE-row accounting against the Act engine's exp rate.
"""

import numpy as np

import concourse.bass as bass
import concourse.tile as tile
from concourse import bacc, mybir
from concourse.bass_utils import run_bass_kernel_spmd
from concourse.masks import make_identity

B, LQ, LKV = 2, 2048, 2048
C, CTX, H, DK = 1024, 768, 16, 64
SCALE = DK ** (-0.5)

F32 = mybir.dt.float32
F16 = mybir.dt.float16

NCC = C // 128       # 8   contraction chunks for Q proj
NCTX = CTX // 128    # 6   contraction chunks for K/V proj
NIT = LQ // 512      # 4   i blocks
NJT = LKV // 128     # 16  j chunks
GROUPS = [(g0, 2) for g0 in range(0, NJT, 2)]   # 8 groups of 2 j-chunks

# filler pacing: target PE rows of filler per pipeline slot (the gap
# between the Act engine's exp time per slot and the attention PE work)
FILL_ROWS_PER_SLOT = 820


def build_kernel():
    nc = bacc.Bacc("TRN2", target_bir_lowering=False, debug=False)

    xT = nc.dram_tensor("xT", [C, LQ], F16, kind="ExternalInput").ap()
    yT = nc.dram_tensor("yT", [CTX, LKV], F16, kind="ExternalInput").ap()
    wq = nc.dram_tensor("wq", [C, 256], F16, kind="ExternalInput").ap()
    wk = nc.dram_tensor("wk", [CTX, 256], F16, kind="ExternalInput").ap()
    wv = nc.dram_tensor("wv", [CTX, 256], F16, kind="ExternalInput").ap()
    wp = nc.dram_tensor("wp", [256, C], F16, kind="ExternalInput").ap()
    outT = nc.dram_tensor("outT", [C, LQ], F32, kind="ExternalOutput").ap()

    xTr = xT.rearrange("(cc p) l -> p cc l", p=128)
    yTr = yT.rearrange("(cc p) l -> p cc l", p=128)
    outTr = outT.rearrange("(ct p) l -> p ct l", p=128)

    with tile.TileContext(nc) as tc:
        with (
            tc.tile_pool(name="prs", bufs=1) as prs,      # persistent SBUF
            tc.tile_pool(name="pt", bufs=4) as ptp,       # exp outputs
            tc.tile_pool(name="nrm", bufs=4) as nrm,      # 1/rowsum
            tc.tile_pool(name="stg", bufs=6) as stg,      # normalized O staging
            tc.tile_pool(name="ob", bufs=2) as obp,       # output staging
            tc.tile_pool(name="st", bufs=2, space="PSUM") as stp,   # scores/proj
            tc.tile_pool(name="ot", bufs=2, space="PSUM") as otp,   # AV accum
        ):
            # ---- persistent SBUF tensors
            x_sb = prs.tile([128, NCC, LQ], F16, tag="x")
            y_sb = prs.tile([128, NCTX, LKV], F16, tag="y")
            wq_sb = prs.tile([128, NCC, 256], F16, tag="wq")
            wk_sb = prs.tile([128, NCTX, 256], F16, tag="wk")
            wv_sb = prs.tile([128, NCTX, 256], F16, tag="wv")
            wp_sb = prs.tile([128, 2, C], F16, tag="wp")
            qt = prs.tile([128, 2, LQ], F16, tag="qt")      # [d(2h), pair, i]
            kt = prs.tile([128, 2, LKV], F16, tag="kt")     # [d(2h), pair, j]
            vaug = prs.tile([128, NJT, 4, 65], F16, tag="va")  # [j, jt, h, d|1]
            otn = prs.tile([128, 2, LQ], F16, tag="otn")    # [hd(2h), pair, i]
            ones = prs.tile([128, NJT, 4], F16, tag="ones")
            ident = prs.tile([128, 128], F16, tag="ident")
            scr = prs.tile([1, 2], F16, tag="scr")          # act-table warm

            # Act Exp table preload + identity for PE transposes (gpsimd)
            nc.vector.memset(scr[:], 0.0)
            nc.scalar.activation(scr[0:1, 0:1], scr[0:1, 1:2],
                                 mybir.ActivationFunctionType.Exp)
            make_identity(nc, ident[:])

            # ---- DMAs (SP queue, ordered to match consumption order)
            nc.sync.dma_start(out=wq_sb, in_=wq.rearrange("(cc p) h -> p cc h", p=128))
            nc.sync.dma_start(out=x_sb[:, 0:4, 0:512], in_=xTr[:, 0:4, 0:512])
            nc.sync.dma_start(out=x_sb[:, 4:8, 0:512], in_=xTr[:, 4:8, 0:512])
            nc.sync.dma_start(out=wk_sb, in_=wk.rearrange("(cc p) h -> p cc h", p=128))
            nc.sync.dma_start(out=y_sb[:, :, 0:512], in_=yTr[:, :, 0:512])
            nc.sync.dma_start(out=wv_sb, in_=wv.rearrange("(cc p) h -> p cc h", p=128))
            for blk in range(1, 4):
                s = slice(blk * 512, (blk + 1) * 512)
                nc.sync.dma_start(out=x_sb[:, :, s], in_=xTr[:, :, s])
                nc.sync.dma_start(out=y_sb[:, :, s], in_=yTr[:, :, s])
            nc.sync.dma_start(out=wp_sb, in_=wp.rearrange("(r p) o -> p r o", p=128))

            nc.vector.memset(ones[:], 1.0)
            nc.vector.tensor_copy(
                vaug[:, :, :, 64:65],
                ones[:].rearrange("p j (h o) -> p j h o", o=1))

            # ---- unit emitters (independent chunks of PE work)
            def q_unit(pair, it):
                ps = stp.tile([128, 512], F32, tag="ps", name="ps_q")
                for cc in range(NCC):
                    nc.tensor.matmul(
                        ps[:], wq_sb[:, cc, pair * 128:(pair + 1) * 128],
                        x_sb[:, cc, it * 512:(it + 1) * 512],
                        start=(cc == 0), stop=(cc == NCC - 1))
                nc.vector.tensor_copy(qt[:, pair, it * 512:(it + 1) * 512], ps[:])

            def k_unit(pair, it):
                ps = stp.tile([128, 512], F32, tag="ps", name="ps_k")
                for cc in range(NCTX):
                    nc.tensor.matmul(
                        ps[:], wk_sb[:, cc, pair * 128:(pair + 1) * 128],
                        y_sb[:, cc, it * 512:(it + 1) * 512],
                        start=(cc == 0), stop=(cc == NCTX - 1))
                nc.vector.tensor_copy(kt[:, pair, it * 512:(it + 1) * 512], ps[:])

            def v_unit(pair, jt):
                # V proj for the 2 heads of `pair`, j-chunk jt
                ps = stp.tile([128, 128], F32, tag="ps", name="ps_v")
                for cc in range(NCTX):
                    nc.tensor.matmul(
                        ps[:], y_sb[:, cc, jt * 128:(jt + 1) * 128],
                        wv_sb[:, cc, pair * 128:(pair + 1) * 128],
                        start=(cc == 0), stop=(cc == NCTX - 1))
                nc.vector.tensor_copy(
                    vaug[:, jt, 2 * pair:2 * pair + 2, 0:64],
                    ps[:].rearrange("p (h d) -> p h d", d=64))

            def d_unit(it, ct, ob):
                # output projection for column-tile ct of i-block it
                ps = stp.tile([128, 512], F32, tag="ps", name="ps_d")
                for pair in range(2):
                    nc.tensor.matmul(
                        ps[:], wp_sb[:, pair, ct * 128:(ct + 1) * 128],
                        otn[:, pair, it * 512:(it + 1) * 512],
                        start=(pair == 0), stop=(pair == 1))
                nc.vector.tensor_copy(ob[:, ct, :], ps[:])

            def t_unit(pair, it, onrms):
                # PE-transpose the 4 normalized [128,128] staging tiles of
                # (pair, it) into otn[:, pair, it-block]
                tp = stp.tile([128, 4, 128], F16, tag="ps", name="ps_t")
                for isub in range(4):
                    for h in range(2):
                        nc.tensor.transpose(
                            tp[64 * h:64 * h + 64, isub, :],
                            onrms[isub][:, 64 * h:64 * h + 64],
                            ident[:])
                nc.vector.tensor_copy(
                    otn[:, pair, it * 512:(it + 1) * 512],
                    tp[:].rearrange("p a b -> p (a b)"))

            # ---- filler scheduling: units are (pe_rows, closure); popped to
            # keep cumulative filler rows at slot_idx * FILL_ROWS_PER_SLOT.
            filler = []
            pending = []   # [slots_left, [(rows, fn), ...]]
            state = {"slots": 0, "rows": 0}

            def push(units, delay=0):
                if delay:
                    pending.append([delay, list(units)])
                else:
                    filler.extend(units)

            def fill(flush=False):
                state["slots"] += 1
                for p in pending:
                    p[0] -= 1
                while pending and pending[0][0] <= 0:
                    filler.extend(pending.pop(0)[1])
                target = state["slots"] * FILL_ROWS_PER_SLOT
                while filler and (flush or state["rows"] < target):
                    rows, fn = filler.pop(0)
                    fn()
                    state["rows"] += rows

            def flush_all():
                while pending or filler:
                    fill(flush=True)

            # ---- attention pipeline pieces
            def scores_exp(pair, it, h, g0, glen):
                d0 = 64 * h
                st = stp.tile([128, glen, 512], F32, tag="st")
                for k in range(glen):
                    jt = g0 + k
                    nc.tensor.matmul(
                        st[:, k, :],
                        kt[d0:d0 + 64, pair, jt * 128:(jt + 1) * 128],
                        qt[d0:d0 + 64, pair, it * 512:(it + 1) * 512],
                        start=True, stop=True)
                pt = ptp.tile([128, glen, 512], F16, tag="pt")
                nc.scalar.activation(
                    pt[:], st[:], mybir.ActivationFunctionType.Exp, scale=SCALE)
                return pt

            ots = {}   # (pair, it) -> (ot_a, ot_b)
            obs = {}   # it -> output staging tile

            def av(pair, it, h, g0, glen, pt):
                if h == 0 and g0 == 0:
                    ot_a = otp.tile([128, 4, 128], F32, tag="ot", name="ot_a")
                    ot_b = otp.tile([128, 4, 128], F32, tag="ot", name="ot_b")
                    ots[(pair, it)] = (ot_a, ot_b)
                ot_h = ots[(pair, it)][h]
                for k in range(glen):
                    jt = g0 + k
                    for isub in range(4):
                        nc.tensor.matmul(
                            ot_h[:, isub, 0:65],
                            pt[:, k, isub * 128:(isub + 1) * 128],
                            vaug[:, jt, 2 * pair + h, :],
                            start=(jt == 0), stop=(jt == NJT - 1))

            def norm(pair, it):
                """DVE: 1/rowsum + scale; returns the 4 staging tiles."""
                ot_a, ot_b = ots.pop((pair, it))
                rs = []
                for h, ot_h in ((0, ot_a), (1, ot_b)):
                    rsinv = nrm.tile([128, 4], F32, tag="rs", name="rsinv")
                    nc.vector.reciprocal(rsinv[:], ot_h[:, :, 64:65])
                    rs.append(rsinv)
                onrms = []
                for isub in range(4):
                    onrm = stg.tile([128, 128], F16, tag="onrm", name="onrm")
                    for h, ot_h in ((0, ot_a), (1, ot_b)):
                        nc.vector.tensor_scalar_mul(
                            onrm[:, 64 * h:64 * h + 64],
                            ot_h[:, isub, 0:64],
                            rs[h][:, isub:isub + 1])
                    onrms.append(onrm)
                return onrms

            def on_it_boundary(pair, it):
                """Called right after the final AV of (pair, it) was emitted."""
                onrms = norm(pair, it)
                # transposes 2 slots later (lets the DVE muls drain first)
                push([(1024, lambda: t_unit(pair, it, onrms))], delay=2)
                if pair == 0 and it == 2:
                    push([(4096, lambda: q_unit(1, 0))])
                if pair == 0 and it == 3:
                    push([(4096, lambda: q_unit(1, 1))])
                if pair == 1:
                    if it + 2 < NIT:
                        push([(4096, lambda: q_unit(1, it + 2))])
                    ob = obp.tile([128, 8, 512], F32, tag="ob", name="ob")
                    obs[it] = ob
                    # D(it) after the transpose unit (+DVE copy) lands
                    push([(1024, lambda ct=ct: d_unit(it, ct, ob))
                          for ct in range(8)], delay=5)
                    if it > 0:
                        nc.sync.dma_start(
                            out=outTr[:, :, (it - 1) * 512:it * 512],
                            in_=obs[it - 1])

            # ---- prefix: minimal pair-0 prerequisites (PE ramps up here)
            q_unit(0, 0)
            k_unit(0, 0)
            for jt in range(4):
                v_unit(0, jt)

            # remaining prep becomes filler, ordered by first consumption
            push([(4096, lambda it=it: q_unit(0, it)) for it in range(1, 4)])
            push([(3072, lambda: k_unit(0, 1))])
            push([(768, lambda jt=jt: v_unit(0, jt)) for jt in range(4, 8)])
            push([(3072, lambda: k_unit(0, 2))])
            push([(768, lambda jt=jt: v_unit(0, jt)) for jt in range(8, 12)])
            push([(3072, lambda: k_unit(0, 3))])
            push([(768, lambda jt=jt: v_unit(0, jt)) for jt in range(12, 16)])
            push([(3072, lambda it=it: k_unit(1, it)) for it in range(4)])
            push([(768, lambda jt=jt: v_unit(1, jt)) for jt in range(16)])

            # ---- main pipeline: scores/exp run 2 slots ahead of AV
            items = [(pair, it, h, g0, glen)
                     for pair in range(2) for it in range(NIT) for h in range(2)
                     for (g0, glen) in GROUPS]
            inflight = []

            def retire(slot_item):
                pair, it, h, g0, glen, pt = slot_item
                av(pair, it, h, g0, glen, pt)
                if h == 1 and g0 + glen == NJT:
                    on_it_boundary(pair, it)

            for item in items:
                pair, it, h, g0, glen = item
                pt = scores_exp(pair, it, h, g0, glen)
                inflight.append((pair, it, h, g0, glen, pt))
                if len(inflight) > 2:
                    retire(inflight.pop(0))
                fill()
            while inflight:
                retire(inflight.pop(0))
                fill()
            flush_all()

            # ---- tail: D(3) + stores (transposes for (1,3) were flushed)
            nc.sync.dma_start(out=outTr[:, :, 2 * 512:3 * 512], in_=obs[2])
            ob3 = obp.tile([128, 8, 512], F32, tag="ob", name="ob3")
            for ct in range(8):
                d_unit(3, ct, ob3)
                if ct % 2 == 1:
                    nc.sync.dma_start(
                        out=outTr[:, ct - 1:ct + 1, 3 * 512:4 * 512],
                        in_=ob3[:, ct - 1:ct + 1, :])

    nc.compile()
    return nc


_NC_CACHE = {}


def _get_nc():
    if "nc" not in _NC_CACHE:
        _NC_CACHE["nc"] = build_kernel()
    return _NC_CACHE["nc"]


def make_in_maps(x, y, Wq, Wkv, Wproj):
    """Host-side sharding: core = b * 4 + hg (hg = 4-head group)."""
    x = np.asarray(x, dtype=np.float32)
    y = np.asarray(y, dtype=np.float32)
    Wq = np.asarray(Wq, dtype=np.float32)
    Wkv = np.asarray(Wkv, dtype=np.float32).reshape(CTX, 2, H, DK)
    Wproj = np.asarray(Wproj, dtype=np.float32)

    in_maps = []
    for core in range(8):
        b, hg = core // 4, core % 4
        hs = slice(4 * hg, 4 * hg + 4)
        in_maps.append({
            "xT": np.ascontiguousarray(x[b].T).astype(np.float16),
            "yT": np.ascontiguousarray(y[b].T).astype(np.float16),
            "wq": np.ascontiguousarray(
                Wq[:, 4 * hg * DK:(4 * hg + 4) * DK]).astype(np.float16),
            "wk": np.ascontiguousarray(
                Wkv[:, 0, hs, :].reshape(CTX, 4 * DK)).astype(np.float16),
            "wv": np.ascontiguousarray(
                Wkv[:, 1, hs, :].reshape(CTX, 4 * DK)).astype(np.float16),
            "wp": np.ascontiguousarray(
                Wproj[4 * hg * DK:(4 * hg + 4) * DK, :]).astype(np.float16),
        })
    return in_maps


def kernel(x, y, Wq, Wkv, Wproj, bproj):
    nc = _get_nc()
    in_maps = make_in_maps(x, y, Wq, Wkv, Wproj)
    res = run_bass_kernel_spmd(nc, in_maps, core_ids=list(range(8)))
    bproj = np.asarray(bproj, dtype=np.float32)
    out = np.empty((B, LQ, C), dtype=np.float32)
    for b in range(B):
        acc = res.results[4 * b]["outT"].astype(np.float32).copy()
        for hg in range(1, 4):
            acc += res.results[4 * b + hg]["outT"]
        out[b] = acc.T + bproj
    return out


# revision 9
# speedup vs baseline: 2.1365x; 1.0025x over previous
"""CrossAttentionBlock kernel for 8 Trainium2 NeuronCores.

Reference computation (per batch b):
    q = x @ Wq;  k,v = y @ Wkv;  per head: softmax(q k^T / sqrt(dk)) v;
    out = concat_heads @ Wproj + bproj

Sharding: 8 cores = 2 batches x 4 head-groups (4 heads each). Each core
computes the partial output contribution of its 4 heads for its batch;
the host sums the 4 partials per batch and adds the bias.

Design notes (cost-model driven):
  - All matmul operands fp16 (1 cycle/row on PE at any size), PSUM fp32.
  - Attention AV is computed in [i, d] orientation (lhsT = P tile), which
    costs 65 rows per j-chunk instead of 512 -> half the PE rows of the
    S^T orientation, and makes the softmax normalization a per-partition
    scalar multiply (no DRAM broadcast bounce). Rowsums come free as a
    65th "ones" column of V.
  - O^T for the output projection comes from PE transposes (fp16, 128
    rows per [128,64] block) collected in a PSUM fp16 tile, then one DVE
    copy per i-block.
  - The PE clock ramps (0.65 -> 1.2 -> 2.4 GHz) and resets on idle, so the
    schedule keeps PE continuously busy: a flat software pipeline over
    (pair, i-block, head, j-group) runs scores 2 slots ahead of the AV
    consumer of the exp output; projection work (Q/K/V of the other pair,
    transposes, output projection) is interleaved as filler units paced
    by PE-row accounting against the Act engine's exp rate.
"""

import numpy as np

import concourse.bass as bass
import concourse.tile as tile
from concourse import bacc, mybir
from concourse.bass_utils import run_bass_kernel_spmd
from concourse.masks import make_identity

B, LQ, LKV = 2, 2048, 2048
C, CTX, H, DK = 1024, 768, 16, 64
SCALE = DK ** (-0.5)

F32 = mybir.dt.float32
F16 = mybir.dt.float16

NCC = C // 128       # 8   contraction chunks for Q proj
NCTX = CTX // 128    # 6   contraction chunks for K/V proj
NIT = LQ // 512      # 4   i blocks
NJT = LKV // 128     # 16  j chunks
GROUPS = [(g0, 2) for g0 in range(0, NJT, 2)]   # 8 groups of 2 j-chunks

# filler pacing: target PE rows of filler per pipeline slot (the gap
# between the Act engine's exp time per slot and the attention PE work)
FILL_ROWS_PER_SLOT = 820


def build_kernel():
    nc = bacc.Bacc("TRN2", target_bir_lowering=False, debug=False)

    xT = nc.dram_tensor("xT", [C, LQ], F16, kind="ExternalInput").ap()
    yT = nc.dram_tensor("yT", [CTX, LKV], F16, kind="ExternalInput").ap()
    wq = nc.dram_tensor("wq", [C, 256], F16, kind="ExternalInput").ap()
    wk = nc.dram_tensor("wk", [CTX, 256], F16, kind="ExternalInput").ap()
    wv = nc.dram_tensor("wv", [CTX, 256], F16, kind="ExternalInput").ap()
    wp = nc.dram_tensor("wp", [256, C], F16, kind="ExternalInput").ap()
    outT = nc.dram_tensor("outT", [C, LQ], F32, kind="ExternalOutput").ap()

    xTr = xT.rearrange("(cc p) l -> p cc l", p=128)
    yTr = yT.rearrange("(cc p) l -> p cc l", p=128)
    outTr = outT.rearrange("(ct p) l -> p ct l", p=128)

    with tile.TileContext(nc) as tc:
        with (
            tc.tile_pool(name="prs", bufs=1) as prs,      # persistent SBUF
            tc.tile_pool(name="pt", bufs=4) as ptp,       # exp outputs
            tc.tile_pool(name="nrm", bufs=4) as nrm,      # 1/rowsum
            tc.tile_pool(name="stg", bufs=6) as stg,      # normalized O staging
            tc.tile_pool(name="ob", bufs=2) as obp,       # output staging
            tc.tile_pool(name="st", bufs=2, space="PSUM") as stp,   # scores/proj
            tc.tile_pool(name="ot", bufs=2, space="PSUM") as otp,   # AV accum
        ):
            # ---- persistent SBUF tensors
            x_sb = prs.tile([128, NCC, LQ], F16, tag="x")
            y_sb = prs.tile([128, NCTX, LKV], F16, tag="y")
            wq_sb = prs.tile([128, NCC, 256], F16, tag="wq")
            wk_sb = prs.tile([128, NCTX, 256], F16, tag="wk")
            wv_sb = prs.tile([128, NCTX, 256], F16, tag="wv")
            wp_sb = prs.tile([128, 2, C], F16, tag="wp")
            qt = prs.tile([128, 2, LQ], F16, tag="qt")      # [d(2h), pair, i]
            kt = prs.tile([128, 2, LKV], F16, tag="kt")     # [d(2h), pair, j]
            vaug = prs.tile([128, NJT, 4, 65], F16, tag="va")  # [j, jt, h, d|1]
            otn = prs.tile([128, 2, LQ], F16, tag="otn")    # [hd(2h), pair, i]
            ones = prs.tile([128, NJT, 4], F16, tag="ones")
            ident = prs.tile([128, 128], F16, tag="ident")
            scr = prs.tile([1, 2], F16, tag="scr")          # act-table warm

            # Act Exp table preload + identity for PE transposes (gpsimd)
            nc.vector.memset(scr[:], 0.0)
            nc.scalar.activation(scr[0:1, 0:1], scr[0:1, 1:2],
                                 mybir.ActivationFunctionType.Exp)
            make_identity(nc, ident[:])

            # ---- DMAs (SP queue, ordered to match consumption order)
            nc.sync.dma_start(out=wq_sb, in_=wq.rearrange("(cc p) h -> p cc h", p=128))
            nc.sync.dma_start(out=x_sb[:, 0:4, 0:512], in_=xTr[:, 0:4, 0:512])
            nc.sync.dma_start(out=x_sb[:, 4:8, 0:512], in_=xTr[:, 4:8, 0:512])
            nc.sync.dma_start(out=wk_sb, in_=wk.rearrange("(cc p) h -> p cc h", p=128))
            nc.sync.dma_start(out=y_sb[:, :, 0:512], in_=yTr[:, :, 0:512])
            nc.sync.dma_start(out=wv_sb, in_=wv.rearrange("(cc p) h -> p cc h", p=128))
            for blk in range(1, 4):
                s = slice(blk * 512, (blk + 1) * 512)
                nc.sync.dma_start(out=x_sb[:, :, s], in_=xTr[:, :, s])
                nc.sync.dma_start(out=y_sb[:, :, s], in_=yTr[:, :, s])
            nc.sync.dma_start(out=wp_sb, in_=wp.rearrange("(r p) o -> p r o", p=128))

            nc.vector.memset(ones[:], 1.0)
            nc.vector.tensor_copy(
                vaug[:, :, :, 64:65],
                ones[:].rearrange("p j (h o) -> p j h o", o=1))

            # ---- unit emitters (independent chunks of PE work)
            def q_unit(pair, it):
                ps = stp.tile([128, 512], F32, tag="ps", name="ps_q")
                for cc in range(NCC):
                    nc.tensor.matmul(
                        ps[:], wq_sb[:, cc, pair * 128:(pair + 1) * 128],
                        x_sb[:, cc, it * 512:(it + 1) * 512],
                        start=(cc == 0), stop=(cc == NCC - 1))
                nc.vector.tensor_copy(qt[:, pair, it * 512:(it + 1) * 512], ps[:])

            def k_unit(pair, it):
                ps = stp.tile([128, 512], F32, tag="ps", name="ps_k")
                for cc in range(NCTX):
                    nc.tensor.matmul(
                        ps[:], wk_sb[:, cc, pair * 128:(pair + 1) * 128],
                        y_sb[:, cc, it * 512:(it + 1) * 512],
                        start=(cc == 0), stop=(cc == NCTX - 1))
                nc.vector.tensor_copy(kt[:, pair, it * 512:(it + 1) * 512], ps[:])

            def v_unit(pair, jt):
                # V proj for the 2 heads of `pair`, j-chunk jt
                ps = stp.tile([128, 128], F32, tag="ps", name="ps_v")
                for cc in range(NCTX):
                    nc.tensor.matmul(
                        ps[:], y_sb[:, cc, jt * 128:(jt + 1) * 128],
                        wv_sb[:, cc, pair * 128:(pair + 1) * 128],
                        start=(cc == 0), stop=(cc == NCTX - 1))
                nc.vector.tensor_copy(
                    vaug[:, jt, 2 * pair:2 * pair + 2, 0:64],
                    ps[:].rearrange("p (h d) -> p h d", d=64))

            def d_unit(it, ct, ob):
                # output projection for column-tile ct of i-block it
                ps = stp.tile([128, 512], F32, tag="ps", name="ps_d")
                for pair in range(2):
                    nc.tensor.matmul(
                        ps[:], wp_sb[:, pair, ct * 128:(ct + 1) * 128],
                        otn[:, pair, it * 512:(it + 1) * 512],
                        start=(pair == 0), stop=(pair == 1))
                nc.vector.tensor_copy(ob[:, ct, :], ps[:])

            def t_unit(pair, it, onrms):
                # PE-transpose the 4 normalized [128,128] staging tiles of
                # (pair, it) into otn[:, pair, it-block]
                tp = stp.tile([128, 4, 128], F16, tag="ps", name="ps_t")
                for isub in range(4):
                    for h in range(2):
                        nc.tensor.transpose(
                            tp[64 * h:64 * h + 64, isub, :],
                            onrms[isub][:, 64 * h:64 * h + 64],
                            ident[:])
                nc.vector.tensor_copy(
                    otn[:, pair, it * 512:(it + 1) * 512],
                    tp[:].rearrange("p a b -> p (a b)"))

            # ---- filler scheduling: units are (pe_rows, key, closure);
            # popped to keep cumulative filler rows at slot_idx *
            # FILL_ROWS_PER_SLOT, or on demand via ensure(key).
            filler = []
            pending = []   # [slots_left, [(rows, key, fn), ...]]
            emitted = set()
            state = {"slots": 0, "rows": 0}

            def push(units, delay=0):
                if delay:
                    pending.append([delay, list(units)])
                else:
                    filler.extend(units)

            def _pop_one():
                rows, key, fn = filler.pop(0)
                fn()
                state["rows"] += rows
                if key:
                    emitted.add(key)

            def ensure(key):
                """Force-emit queued filler up to (and incl.) `key`."""
                while key not in emitted and any(u[1] == key for u in filler):
                    _pop_one()

            def fill(flush=False):
                state["slots"] += 1
                for p in pending:
                    p[0] -= 1
                while pending and pending[0][0] <= 0:
                    filler.extend(pending.pop(0)[1])
                target = state["slots"] * FILL_ROWS_PER_SLOT
                while filler and (flush or state["rows"] < target):
                    _pop_one()

            def flush_all():
                while pending or filler:
                    fill(flush=True)

            # ---- attention pipeline pieces
            def scores_exp(pair, it, h, g0, glen):
                ensure(("q", pair, it))
                for blk in range(g0 // 4, (g0 + glen - 1) // 4 + 1):
                    ensure(("k", pair, blk))
                d0 = 64 * h
                st = stp.tile([128, glen, 512], F32, tag="st")
                for k in range(glen):
                    jt = g0 + k
                    nc.tensor.matmul(
                        st[:, k, :],
                        kt[d0:d0 + 64, pair, jt * 128:(jt + 1) * 128],
                        qt[d0:d0 + 64, pair, it * 512:(it + 1) * 512],
                        start=True, stop=True)
                pt = ptp.tile([128, glen, 512], F16, tag="pt")
                nc.scalar.activation(
                    pt[:], st[:], mybir.ActivationFunctionType.Exp, scale=SCALE)
                return pt

            ots = {}   # (pair, it) -> (ot_a, ot_b)
            obs = {}   # it -> output staging tile

            def av(pair, it, h, g0, glen, pt):
                for jt in range(g0, g0 + glen):
                    ensure(("v", pair, jt))
                if h == 0 and g0 == 0:
                    ot_a = otp.tile([128, 4, 128], F32, tag="ot", name="ot_a")
                    ot_b = otp.tile([128, 4, 128], F32, tag="ot", name="ot_b")
                    ots[(pair, it)] = (ot_a, ot_b)
                ot_h = ots[(pair, it)][h]
                for k in range(glen):
                    jt = g0 + k
                    for isub in range(4):
                        nc.tensor.matmul(
                            ot_h[:, isub, 0:65],
                            pt[:, k, isub * 128:(isub + 1) * 128],
                            vaug[:, jt, 2 * pair + h, :],
                            start=(jt == 0), stop=(jt == NJT - 1))

            def norm(pair, it):
                """DVE: 1/rowsum + scale; returns the 4 staging tiles."""
                ot_a, ot_b = ots.pop((pair, it))
                rs = []
                for h, ot_h in ((0, ot_a), (1, ot_b)):
                    rsinv = nrm.tile([128, 4], F32, tag="rs", name="rsinv")
                    nc.vector.reciprocal(rsinv[:], ot_h[:, :, 64:65])
                    rs.append(rsinv)
                onrms = []
                for isub in range(4):
                    onrm = stg.tile([128, 128], F16, tag="onrm", name="onrm")
                    for h, ot_h in ((0, ot_a), (1, ot_b)):
                        nc.vector.tensor_scalar_mul(
                            onrm[:, 64 * h:64 * h + 64],
                            ot_h[:, isub, 0:64],
                            rs[h][:, isub:isub + 1])
                    onrms.append(onrm)
                return onrms

            def on_it_boundary(pair, it):
                """Called right after the final AV of (pair, it) was emitted."""
                onrms = norm(pair, it)
                # transposes 2 slots later (lets the DVE muls drain first)
                push([(1024, None, lambda: t_unit(pair, it, onrms))], delay=2)
                if pair == 0 and it == 2:
                    push([(4096, ("q", 1, 0), lambda: q_unit(1, 0))])
                if pair == 0 and it == 3:
                    push([(4096, ("q", 1, 1), lambda: q_unit(1, 1))])
                if pair == 1:
                    if it + 2 < NIT:
                        push([(4096, ("q", 1, it + 2),
                               lambda: q_unit(1, it + 2))])
                    ob = obp.tile([128, 8, 512], F32, tag="ob", name="ob")
                    obs[it] = ob
                    # D(it) after the transpose unit (+DVE copy) lands
                    push([(1024, None, lambda ct=ct: d_unit(it, ct, ob))
                          for ct in range(8)], delay=5)
                    if it > 0:
                        nc.sync.dma_start(
                            out=outTr[:, :, (it - 1) * 512:it * 512],
                            in_=obs[it - 1])

            # ---- prefix: minimal pair-0 prerequisites (PE ramps up here)
            q_unit(0, 0)
            k_unit(0, 0)
            for jt in range(4):
                v_unit(0, jt)

            # remaining prep becomes filler, ordered by first consumption
            push([(4096, ("q", 0, it), lambda it=it: q_unit(0, it))
                  for it in range(1, 4)])
            push([(3072, ("k", 0, 1), lambda: k_unit(0, 1))])
            push([(768, ("v", 0, jt), lambda jt=jt: v_unit(0, jt))
                  for jt in range(4, 8)])
            push([(3072, ("k", 0, 2), lambda: k_unit(0, 2))])
            push([(768, ("v", 0, jt), lambda jt=jt: v_unit(0, jt))
                  for jt in range(8, 12)])
            push([(3072, ("k", 0, 3), lambda: k_unit(0, 3))])
            push([(768, ("v", 0, jt), lambda jt=jt: v_unit(0, jt))
                  for jt in range(12, 16)])
            push([(3072, ("k", 1, it), lambda it=it: k_unit(1, it))
                  for it in range(4)])
            push([(768, ("v", 1, jt), lambda jt=jt: v_unit(1, jt))
                  for jt in range(16)])
            emitted.update({("q", 0, 0), ("k", 0, 0)}
                           | {("v", 0, jt) for jt in range(4)})

            # ---- main pipeline: scores/exp run 2 slots ahead of AV
            items = [(0, 0, h, g0, glen) for (g0, glen) in GROUPS
                     for h in range(2)]
            items += [(pair, it, h, g0, glen)
                      for pair in range(2) for it in range(NIT)
                      for h in range(2) for (g0, glen) in GROUPS
                      if not (pair == 0 and it == 0)]
            inflight = []

            def retire(slot_item):
                pair, it, h, g0, glen, pt = slot_item
                av(pair, it, h, g0, glen, pt)
                if h == 1 and g0 + glen == NJT:
                    on_it_boundary(pair, it)

            for item in items:
                pair, it, h, g0, glen = item
                pt = scores_exp(pair, it, h, g0, glen)
                inflight.append((pair, it, h, g0, glen, pt))
                if len(inflight) > 2:
                    retire(inflight.pop(0))
                fill()
            while inflight:
                retire(inflight.pop(0))
                fill()
            flush_all()

            # ---- tail: D(3) + stores (transposes for (1,3) were flushed)
            nc.sync.dma_start(out=outTr[:, :, 2 * 512:3 * 512], in_=obs[2])
            ob3 = obp.tile([128, 8, 512], F32, tag="ob", name="ob3")
            for ct in range(8):
                d_unit(3, ct, ob3)
                if ct % 2 == 1:
                    nc.sync.dma_start(
                        out=outTr[:, ct - 1:ct + 1, 3 * 512:4 * 512],
                        in_=ob3[:, ct - 1:ct + 1, :])

    nc.compile()
    return nc


_NC_CACHE = {}


def _get_nc():
    if "nc" not in _NC_CACHE:
        _NC_CACHE["nc"] = build_kernel()
    return _NC_CACHE["nc"]


def make_in_maps(x, y, Wq, Wkv, Wproj):
    """Host-side sharding: core = b * 4 + hg (hg = 4-head group)."""
    x = np.asarray(x, dtype=np.float32)
    y = np.asarray(y, dtype=np.float32)
    Wq = np.asarray(Wq, dtype=np.float32)
    Wkv = np.asarray(Wkv, dtype=np.float32).reshape(CTX, 2, H, DK)
    Wproj = np.asarray(Wproj, dtype=np.float32)

    in_maps = []
    for core in range(8):
        b, hg = core // 4, core % 4
        hs = slice(4 * hg, 4 * hg + 4)
        in_maps.append({
            "xT": np.ascontiguousarray(x[b].T).astype(np.float16),
            "yT": np.ascontiguousarray(y[b].T).astype(np.float16),
            "wq": np.ascontiguousarray(
                Wq[:, 4 * hg * DK:(4 * hg + 4) * DK]).astype(np.float16),
            "wk": np.ascontiguousarray(
                Wkv[:, 0, hs, :].reshape(CTX, 4 * DK)).astype(np.float16),
            "wv": np.ascontiguousarray(
                Wkv[:, 1, hs, :].reshape(CTX, 4 * DK)).astype(np.float16),
            "wp": np.ascontiguousarray(
                Wproj[4 * hg * DK:(4 * hg + 4) * DK, :]).astype(np.float16),
        })
    return in_maps


def kernel(x, y, Wq, Wkv, Wproj, bproj):
    nc = _get_nc()
    in_maps = make_in_maps(x, y, Wq, Wkv, Wproj)
    res = run_bass_kernel_spmd(nc, in_maps, core_ids=list(range(8)))
    bproj = np.asarray(bproj, dtype=np.float32)
    out = np.empty((B, LQ, C), dtype=np.float32)
    for b in range(B):
        acc = res.results[4 * b]["outT"].astype(np.float32).copy()
        for hg in range(1, 4):
            acc += res.results[4 * b + hg]["outT"]
        out[b] = acc.T + bproj
    return out
